# revision 1
# baseline (speedup 1.0000x reference)
"""Multi-head attention forward (B=4,T=2048,C=1024,H=16 causal) on 8 TRN2
NeuronCores via Bass/Tile.

Sharding: batch x head-group. Core c handles batch b=c//2 and heads
[g*8,(g+1)*8) where g=c%2. Each core computes its QKV projections
(column-sharded), causal attention for its 8 heads, and a row-sharded
partial of the output projection. The host sums the two partials per
batch and adds the bias.

Device layouts (T=2048, C=1024, HD=512 local head-dims):
  - scores are computed transposed (k on partitions, q free) so the
    softmax denominator falls out of the attn@V matmul via a ones-column
    appended to V ("vext"), and causal masking is a PSUM seed matmul of
    a constant bias tile (start=True) that the scores accumulate onto.
  - normalization: reciprocal of the sums row, gpsimd partition-broadcast,
    one vector multiply. No transposes anywhere on device.
"""
import sys
sys.path.insert(0, '/opt/trn_rl_repo')

import numpy as np
import ml_dtypes

B, T, C, H, D = 4, 2048, 1024, 16, 64
HPC, HD = 8, 512            # heads per core, local head-dim total
NEG = -30000.0

_CACHE = {}


def _build():
    from contextlib import ExitStack
    import concourse.bacc as bacc
    import concourse.tile as tile
    from concourse import mybir

    f32 = mybir.dt.float32
    f32r = mybir.dt.float32r
    bf16 = mybir.dt.bfloat16
    EXP = mybir.ActivationFunctionType.Exp

    nc = bacc.Bacc("TRN2", target_bir_lowering=False, debug=False, num_devices=1)

    xq_d = nc.dram_tensor("xqT", [C, T], bf16, kind="ExternalInput").ap()
    xk_d = nc.dram_tensor("xkT", [C, T], bf16, kind="ExternalInput").ap()
    xv_d = nc.dram_tensor("xvT", [C, T], bf16, kind="ExternalInput").ap()
    wq_d = nc.dram_tensor("wq", [C, HD], bf16, kind="ExternalInput").ap()
    wk_d = nc.dram_tensor("wk", [C, HD], bf16, kind="ExternalInput").ap()
    wv_d = nc.dram_tensor("wv", [C, HD], bf16, kind="ExternalInput").ap()
    wo_d = nc.dram_tensor("wo", [HD, C], bf16, kind="ExternalInput").ap()
    cd_d = nc.dram_tensor("cdiag", [128, 128], bf16, kind="ExternalInput").ap()
    y_d = nc.dram_tensor("y", [T, C], f32, kind="ExternalOutput").ap()

    with tile.TileContext(nc) as tc, ExitStack() as ctx:
        pw = ctx.enter_context(tc.tile_pool(name="pw", bufs=1))
        pqts = ctx.enter_context(tc.tile_pool(name="pqts", bufs=3))
        pkts = ctx.enter_context(tc.tile_pool(name="pkts", bufs=4))
        pvext = ctx.enter_context(tc.tile_pool(name="pvext", bufs=16))
        pctxn = ctx.enter_context(tc.tile_pool(name="pctxn", bufs=2))
        px = ctx.enter_context(tc.tile_pool(name="px", bufs=8))
        pex = ctx.enter_context(tc.tile_pool(name="pex", bufs=12))
        pr = ctx.enter_context(tc.tile_pool(name="pr", bufs=4))
        pys = ctx.enter_context(tc.tile_pool(name="pys", bufs=3))
        pps = ctx.enter_context(tc.tile_pool(name="pps", bufs=8, space="PSUM"))

        # ---- constants + resident weights (wq first so proj_q(0) can
        # start compute while the rest stream in)
        cd = pw.tile([128, 128], bf16, tag="cd")
        nc.sync.dma_start(cd[:], cd_d[:])
        wq_s = pw.tile([128, 8, HD], bf16, tag="wq")
        wk_s = pw.tile([128, 8, HD], bf16, tag="wk")
        wv_s = pw.tile([128, 8, HD], bf16, tag="wv")
        wo_s = pw.tile([128, 4, C], bf16, tag="wo")

        def load_w(w_s, w_src):
            for ct in range(8):
                nc.sync.dma_start(w_s[:, ct, :],
                                  w_src[ct * 128:(ct + 1) * 128, :])

        kts = [None] * 4     # kT window tiles [128, 4, 512]
        vext = [None] * 16   # vext chunk tiles [128, 8, 65] bf16

        def load_x2(x_src, cp, t4):
            """One 512KB DMA: c-tiles 2cp,2cp+1 of window t4 -> [128,2,512]."""
            x2 = px.tile([128, 2, 512], bf16, tag="x", name="x2")
            nc.sync.dma_start(
                x2[:],
                x_src[2 * cp * 128:(2 * cp + 2) * 128,
                      t4 * 512:(t4 + 1) * 512].rearrange(
                          "(two p) t -> p two t", p=128))
            return x2

        def proj_qk(w_s, x_src, t4, tag):
            """qT/kT window: out[pair-row, hp, t] for t in window t4."""
            ps = [pps.tile([128, 512], f32, tag="ps", name=f"ps{i}") for i in range(4)]
            for cp in range(4):
                x2 = load_x2(x_src, cp, t4)
                for half in range(2):
                    ct = 2 * cp + half
                    for j in range(4):
                        nc.tensor.matmul(
                            ps[j][:],
                            lhsT=w_s[:, ct, j * 128:(j + 1) * 128],
                            rhs=x2[:, half, :],
                            start=(ct == 0), stop=(ct == 7))
            dst = (pqts if tag == "qts" else pkts).tile(
                [128, 4, 512], bf16, tag=tag, name=tag)
            for j in range(4):
                nc.vector.tensor_copy(dst[:, j, :], ps[j][:])
            return dst

        def proj_v(t4):
            """v chunks: vext[kc][p=t%128, h, 0:64]=v, [..,64]=1."""
            ps = [pps.tile([128, 512], f32, tag="ps", name=f"ps{i}") for i in range(4)]
            for cp in range(4):
                x2 = load_x2(xv_d, cp, t4)
                for half in range(2):
                    ct = 2 * cp + half
                    for tc4 in range(4):
                        nc.tensor.matmul(
                            ps[tc4][:],
                            lhsT=x2[:, half, tc4 * 128:(tc4 + 1) * 128],
                            rhs=wv_s[:, ct, :],
                            start=(ct == 0), stop=(ct == 7))
            for tc4 in range(4):
                kc = 4 * t4 + tc4
                vx = pvext.tile([128, 8, 65], bf16, tag="vext", name="vx")
                nc.vector.tensor_copy(
                    vx[:, :, 0:64],
                    ps[tc4][:].rearrange("p (h d) -> p h d", h=8))
                nc.gpsimd.memset(vx[:, :, 64:65], 1.0)
                vext[kc] = vx

        def attention_hp(qt, qts, ctxn, hp):
            nki = 4 * qt + 4
            if True:
                ctx2 = [pps.tile([65, 512], f32, tag="ps", name=f"ctx{i}") for i in range(2)]
                pending = []   # ctx matmuls delayed one k-tile (SW pipeline)
                for ki in range(nki):
                    cur = []
                    for hh in range(2):
                        h = 2 * hp + hh
                        pb = hh * 64
                        diag = (ki // 4 == qt)
                        off = (ki % 4) * 128 if diag else 0
                        sT = pps.tile([128, 512], f32, tag="ps", name="sT")
                        ks = kts[ki // 4][pb:pb + 64, hp,
                                          (ki % 4) * 128:(ki % 4 + 1) * 128]
                        qs = qts[pb:pb + 64, hp, off:512]
                        nc.tensor.matmul(sT[:, off:], lhsT=ks, rhs=qs,
                                         start=True, stop=True)
                        ex = pex.tile([128, 512], bf16, tag="ex", name="ex")
                        nc.scalar.activation(ex[:, off:], sT[:, off:], EXP,
                                             scale=0.125)
                        if diag:
                            # zero the dead (k>q) triangle+cols of this block
                            nc.vector.tensor_mul(ex[:, off:off + 128],
                                                 ex[:, off:off + 128], cd[:])
                        cur.append((hh, h, off, ex, ki))
                    for (phh, ph, poff, pex_t, pki) in pending:
                        nc.tensor.matmul(
                            ctx2[phh][:, poff:], lhsT=vext[pki][:, ph, :],
                            rhs=pex_t[:, poff:],
                            start=(pki == 0), stop=(pki == nki - 1))
                    pending = cur
                for (phh, ph, poff, pex_t, pki) in pending:
                    nc.tensor.matmul(
                        ctx2[phh][:, poff:], lhsT=vext[pki][:, ph, :],
                        rhs=pex_t[:, poff:],
                        start=(pki == 0), stop=(pki == nki - 1))
                for hh in range(2):
                    srow = pr.tile([1, 512], f32, tag="srow", name="srow")
                    nc.vector.tensor_copy(srow[:], ctx2[hh][64:65, :])
                    rrow = pr.tile([1, 512], f32, tag="rrow", name="rrow")
                    nc.vector.reciprocal_approx_fast(rrow[:], srow[:])
                    rb = pr.tile([64, 512], f32, tag="rb", name="rb")
                    nc.gpsimd.partition_broadcast(rb[:], rrow[:])
                    if hh == 0:
                        nc.vector.tensor_mul(ctxn[0:64, hp, :],
                                             ctx2[hh][0:64, :], rb[:])
                    else:
                        tmp = pr.tile([64, 512], bf16, tag="tmp", name="tmp")
                        nc.vector.tensor_mul(tmp[:], ctx2[hh][0:64, :], rb[:])
                        nc.sync.dma_start(ctxn[64:128, hp, :], tmp[:])

        def outproj_chunk(qt, ctxn, qc4, ch, split=None):
            if split is None:
                yp = pps.tile([128, 512], f32, tag="ps", name="yp")
                js = range(4)
            else:
                yp = split
                js = (3,)
            for j in js:
                nc.tensor.matmul(
                    yp[:],
                    lhsT=ctxn[:, j, qc4 * 128:(qc4 + 1) * 128],
                    rhs=wo_s[:, j, ch * 512:(ch + 1) * 512],
                    start=(j == 0), stop=(j == 3))
            ys = pys.tile([128, 512], f32, tag="ys", name="ys")
            nc.vector.tensor_copy(ys[:], yp[:])
            nc.sync.dma_start(
                y_d[(qt * 4 + qc4) * 128:(qt * 4 + qc4 + 1) * 128,
                    ch * 512:(ch + 1) * 512], ys[:])


        # Interleave: attention(t4) hp-blocks carry next window's
        # projections and the previous window's out-projection on the
        # PE, keeping it dense (HAM warm) while ScalarE streams exps.
        # warm the ACT exp table during initial DMA
        warm = pr.tile([1, 8], f32, tag="warm", name="warm")
        nc.gpsimd.memset(warm[:], 0.0)
        nc.scalar.activation(warm[:], warm[:], EXP, scale=1.0)
        load_w(wq_s, wq_d)
        qts_cur = proj_qk(wq_s, xq_d, 0, "qts")
        load_w(wk_s, wk_d)
        kts[0] = proj_qk(wk_s, xk_d, 0, "kts")
        load_w(wv_s, wv_d)
        proj_v(0)
        for j in range(4):
            nc.sync.dma_start(wo_s[:, j, :], wo_d[j * 128:(j + 1) * 128, :])
        prev_ctxn = None
        for t4 in range(4):
            ctxn = pctxn.tile([128, 4, 512], bf16, tag="ctxn", name="ctxn")
            qts_next = None
            for hp in range(4):
                attention_hp(t4, qts_cur, ctxn, hp)
                if t4 < 3:
                    if hp == 0:
                        qts_next = proj_qk(wq_s, xq_d, t4 + 1, "qts")
                    elif hp == 1:
                        kts[t4 + 1] = proj_qk(wk_s, xk_d, t4 + 1, "kts")
                    elif hp == 2:
                        proj_v(t4 + 1)
                if prev_ctxn is not None:
                    # spread the previous window's out-projection: 2 of
                    # its 8 chunks after each hp block
                    for c in range(2):
                        idx = hp * 2 + c
                        outproj_chunk(t4 - 1, prev_ctxn, idx // 2, idx % 2)
            prev_ctxn = ctxn
            qts_cur = qts_next
        for qc4 in range(4):
            for ch in range(2):
                outproj_chunk(3, prev_ctxn, qc4, ch)

    nc.compile()
    return nc


def _numpy_fallback(query, key, value, mask, causal_mask, Wq, Wk, Wv, Wo, bo):
    q = (query @ Wq.T).reshape(B, T, H, D).transpose(0, 2, 1, 3)
    k = (key @ Wk.T).reshape(B, T, H, D).transpose(0, 2, 1, 3)
    v = (value @ Wv.T).reshape(B, T, H, D).transpose(0, 2, 1, 3)
    out = np.zeros((B, H, T, D), np.float32)
    for b in range(B):
        for h in range(H):
            s = (q[b, h] @ k[b, h].T) / np.sqrt(np.float32(D))
            s = np.where(mask[b, 0, 0][None, :] == 0, -np.inf, s)
            if causal_mask:
                tri = np.tril(np.ones((T, T), bool))
                s = np.where(tri, s, -np.inf)
            s = s - s.max(axis=-1, keepdims=True)
            e = np.exp(s)
            a = e / e.sum(axis=-1, keepdims=True)
            out[b, h] = a @ v[b, h]
    out = out.transpose(0, 2, 1, 3).reshape(B, T, C)
    return out @ Wo.T + bo


def kernel(**inputs):
    from concourse import bass_utils

    inp = {k: np.asarray(v) for k, v in inputs.items()}
    query, key, value = inp["query"], inp["key"], inp["value"]
    Wq, Wk, Wv, Wo, bo = inp["Wq"], inp["Wk"], inp["Wv"], inp["Wo"], inp["bo"]
    mask, causal_mask = inp["mask"], int(inp["causal_mask"])

    if (mask == 0).any() or causal_mask != 1:
        return _numpy_fallback(
            query.astype(np.float32), key.astype(np.float32),
            value.astype(np.float32), mask, causal_mask,
            Wq.astype(np.float32), Wk.astype(np.float32),
            Wv.astype(np.float32), Wo.astype(np.float32),
            bo.astype(np.float32))

    if "nc" not in _CACHE:
        _CACHE["nc"] = _build()
    nc = _CACHE["nc"]

    cdiag = (np.arange(128)[:, None] <= np.arange(128)[None, :]
             ).astype(ml_dtypes.bfloat16)

    in_maps = []
    for core in range(8):
        b, g = core // 2, core % 2
        hs = g * HD
        in_maps.append({
            "xqT": np.ascontiguousarray(query[b].T).astype(ml_dtypes.bfloat16),
            "xkT": np.ascontiguousarray(key[b].T).astype(ml_dtypes.bfloat16),
            "xvT": np.ascontiguousarray(value[b].T).astype(ml_dtypes.bfloat16),
            "wq": np.ascontiguousarray(Wq[hs:hs + HD, :].T).astype(ml_dtypes.bfloat16),
            "wk": np.ascontiguousarray(Wk[hs:hs + HD, :].T).astype(ml_dtypes.bfloat16),
            "wv": np.ascontiguousarray(Wv[hs:hs + HD, :].T).astype(ml_dtypes.bfloat16),
            "wo": np.ascontiguousarray(Wo[:, hs:hs + HD].T).astype(ml_dtypes.bfloat16),
            "cdiag": cdiag,
        })

    res = bass_utils.run_bass_kernel_spmd(nc, in_maps, core_ids=list(range(8)))
    out = np.zeros((B, T, C), np.float32)
    for core in range(8):
        out[core // 2] += res.results[core]["y"]
    out += bo.astype(np.float32)
    return out


def run_traced(tmpdir=None, **inputs):
    """Profiled run (test harness helper): returns BassKernelResults with
    exec_time_ns/trace populated when the axon NTFF hook is available."""
    from concourse import bass_utils

    inp = {k: np.asarray(v) for k, v in inputs.items()}
    if "nc" not in _CACHE:
        _CACHE["nc"] = _build()
    nc = _CACHE["nc"]
    query, key, value = inp["query"], inp["key"], inp["value"]
    Wq, Wk, Wv, Wo = inp["Wq"], inp["Wk"], inp["Wv"], inp["Wo"]
    cdiag = (np.arange(128)[:, None] <= np.arange(128)[None, :]
             ).astype(ml_dtypes.bfloat16)
    in_maps = []
    for core in range(8):
        b, g = core // 2, core % 2
        hs = g * HD
        in_maps.append({
            "xqT": np.ascontiguousarray(query[b].T).astype(ml_dtypes.bfloat16),
            "xkT": np.ascontiguousarray(key[b].T).astype(ml_dtypes.bfloat16),
            "xvT": np.ascontiguousarray(value[b].T).astype(ml_dtypes.bfloat16),
            "wq": np.ascontiguousarray(Wq[hs:hs + HD, :].T).astype(ml_dtypes.bfloat16),
            "wk": np.ascontiguousarray(Wk[hs:hs + HD, :].T).astype(ml_dtypes.bfloat16),
            "wv": np.ascontiguousarray(Wv[hs:hs + HD, :].T).astype(ml_dtypes.bfloat16),
            "wo": np.ascontiguousarray(Wo[:, hs:hs + HD].T).astype(ml_dtypes.bfloat16),
            "cdiag": cdiag,
        })
    return bass_utils.run_bass_kernel_spmd(
        nc, in_maps, core_ids=list(range(8)), trace=True, tmpdir=tmpdir)



# revision 2
# speedup vs baseline: 1.0222x; 1.0222x over previous
"""Multi-head attention forward (B=4,T=2048,C=1024,H=16 causal) on 8 TRN2
NeuronCores via Bass/Tile.

Sharding: batch x head-group. Core c handles batch b=c//2 and heads
[g*8,(g+1)*8) where g=c%2. Each core computes its QKV projections
(column-sharded), causal attention for its 8 heads, and a row-sharded
partial of the output projection. The host sums the two partials per
batch and adds the bias.

Device layouts (T=2048, C=1024, HD=512 local head-dims):
  - scores are computed transposed (k on partitions, q free) so the
    softmax denominator falls out of the attn@V matmul via a ones-column
    appended to V ("vext").
  - the two heads of an hp-pair live in partitions 0-63 / 64-127, so
    their K=64 score matmuls row-tile into the PE array concurrently
    (tile_position (0,0) / (64,0)) and a single wide ACTIVATE
    exponentiates both heads' scores from a 2-bank PSUM tile.
  - normalization: reciprocal of the sums row, gpsimd partition-broadcast,
    one vector multiply. No transposes anywhere on device.
"""
import sys
sys.path.insert(0, '/opt/trn_rl_repo')

import numpy as np
import ml_dtypes

B, T, C, H, D = 4, 2048, 1024, 16, 64
HPC, HD = 8, 512            # heads per core, local head-dim total
NEG = -30000.0

_CACHE = {}


def _build():
    from contextlib import ExitStack
    import concourse.bacc as bacc
    import concourse.tile as tile
    from concourse import mybir

    f32 = mybir.dt.float32
    bf16 = mybir.dt.bfloat16
    EXP = mybir.ActivationFunctionType.Exp

    nc = bacc.Bacc("TRN2", target_bir_lowering=False, debug=False, num_devices=1)

    xq_d = nc.dram_tensor("xqT", [C, T], bf16, kind="ExternalInput").ap()
    xk_d = nc.dram_tensor("xkT", [C, T], bf16, kind="ExternalInput").ap()
    xv_d = nc.dram_tensor("xvT", [C, T], bf16, kind="ExternalInput").ap()
    wq_d = nc.dram_tensor("wq", [C, HD], bf16, kind="ExternalInput").ap()
    wk_d = nc.dram_tensor("wk", [C, HD], bf16, kind="ExternalInput").ap()
    wv_d = nc.dram_tensor("wv", [C, HD], bf16, kind="ExternalInput").ap()
    wo_d = nc.dram_tensor("wo", [HD, C], bf16, kind="ExternalInput").ap()
    cd_d = nc.dram_tensor("cdiag", [128, 256], bf16, kind="ExternalInput").ap()
    y_d = nc.dram_tensor("y", [T, C], f32, kind="ExternalOutput").ap()

    with tile.TileContext(nc) as tc, ExitStack() as ctx:
        pw = ctx.enter_context(tc.tile_pool(name="pw", bufs=1))
        pqts = ctx.enter_context(tc.tile_pool(name="pqts", bufs=3))
        pkts = ctx.enter_context(tc.tile_pool(name="pkts", bufs=4))
        pvext = ctx.enter_context(tc.tile_pool(name="pvext", bufs=16))
        pctxn = ctx.enter_context(tc.tile_pool(name="pctxn", bufs=2))
        px = ctx.enter_context(tc.tile_pool(name="px", bufs=8))
        pex = ctx.enter_context(tc.tile_pool(name="pex", bufs=8))
        pr = ctx.enter_context(tc.tile_pool(name="pr", bufs=4))
        pys = ctx.enter_context(tc.tile_pool(name="pys", bufs=3))
        pps = ctx.enter_context(tc.tile_pool(name="pps", bufs=1, space="PSUM"))

        # ---- PE warmup: keep HAM busy during the initial weight DMAs so
        # real matmuls start at 2.4 GHz.
        warmT = pw.tile([128, 128], bf16, tag="warmT")
        nc.gpsimd.memset(warmT[:], 0.0)
        wps = pps.tile([128, 512], f32, tag="ps", bufs=2, name="wps")
        for _ in range(40):
            nc.tensor.matmul(wps[:, 0:128], lhsT=warmT[:], rhs=warmT[:],
                             start=True, stop=True)

        # ---- constants + resident weights (wq first so proj_q(0) can
        # start compute while the rest stream in)
        cd2 = pw.tile([128, 2, 128], bf16, tag="cd2")
        nc.sync.dma_start(cd2[:], cd_d[:].rearrange("p (two q) -> p two q", two=2))
        wq_s = pw.tile([128, 8, HD], bf16, tag="wq")
        wk_s = pw.tile([128, 8, HD], bf16, tag="wk")
        wv_s = pw.tile([128, 8, HD], bf16, tag="wv")
        wo_s = pw.tile([128, 4, C], bf16, tag="wo")

        def load_w(w_s, w_src):
            for ct in range(8):
                nc.sync.dma_start(w_s[:, ct, :],
                                  w_src[ct * 128:(ct + 1) * 128, :])

        kts = [None] * 4     # kT window tiles [128, 4, 512]
        vext = [None] * 16   # vext chunk tiles [128, 8, 65] bf16

        def load_x2(x_src, cp, t4):
            """One 512KB DMA: c-tiles 2cp,2cp+1 of window t4 -> [128,2,512]."""
            x2 = px.tile([128, 2, 512], bf16, tag="x", name="x2")
            nc.sync.dma_start(
                x2[:],
                x_src[2 * cp * 128:(2 * cp + 2) * 128,
                      t4 * 512:(t4 + 1) * 512].rearrange(
                          "(two p) t -> p two t", p=128))
            return x2

        def proj_qk(w_s, x_src, t4, tag):
            """qT/kT window: out[pair-row, hp, t] for t in window t4.

            Two passes of 2 PSUM banks each (j 0,1 then 2,3); the x2
            tiles stay resident in SBUF so pass B re-reads them."""
            dst = (pqts if tag == "qts" else pkts).tile(
                [128, 4, 512], bf16, tag=tag, name=tag)
            x2s = []
            ps = [pps.tile([128, 512], f32, tag="ps", bufs=2,
                           name=f"ps{i}") for i in range(2)]
            for cp in range(4):
                x2 = load_x2(x_src, cp, t4)
                x2s.append(x2)
                for half in range(2):
                    ct = 2 * cp + half
                    for j in range(2):
                        nc.tensor.matmul(
                            ps[j][:],
                            lhsT=w_s[:, ct, j * 128:(j + 1) * 128],
                            rhs=x2[:, half, :],
                            start=(ct == 0), stop=(ct == 7))
            for j in range(2):
                nc.vector.tensor_copy(dst[:, j, :], ps[j][:])
            ps2 = [pps.tile([128, 512], f32, tag="ps", bufs=2,
                            name=f"ps{i+2}") for i in range(2)]
            for cp in range(4):
                for half in range(2):
                    ct = 2 * cp + half
                    for j in range(2):
                        nc.tensor.matmul(
                            ps2[j][:],
                            lhsT=w_s[:, ct, (j + 2) * 128:(j + 3) * 128],
                            rhs=x2s[cp][:, half, :],
                            start=(ct == 0), stop=(ct == 7))
            for j in range(2):
                nc.vector.tensor_copy(dst[:, j + 2, :], ps2[j][:])
            return dst

        def proj_v(t4):
            """v chunks: vext[kc][p=t%128, h, 0:64]=v, [..,64]=1.

            Same 2-bank two-pass structure (t-chunks 0,1 then 2,3)."""
            x2s = []
            ps = [pps.tile([128, 512], f32, tag="ps", bufs=2,
                           name=f"vps{i}") for i in range(2)]
            for cp in range(4):
                x2 = load_x2(xv_d, cp, t4)
                x2s.append(x2)
                for half in range(2):
                    ct = 2 * cp + half
                    for tc4 in range(2):
                        nc.tensor.matmul(
                            ps[tc4][:],
                            lhsT=x2[:, half, tc4 * 128:(tc4 + 1) * 128],
                            rhs=wv_s[:, ct, :],
                            start=(ct == 0), stop=(ct == 7))
            def drain(ps_pair, base):
                for i in range(2):
                    kc = 4 * t4 + base + i
                    vx = pvext.tile([128, 8, 65], bf16, tag="vext", name="vx")
                    nc.vector.tensor_copy(
                        vx[:, :, 0:64],
                        ps_pair[i][:].rearrange("p (h d) -> p h d", h=8))
                    nc.gpsimd.memset(vx[:, :, 64:65], 1.0)
                    vext[kc] = vx
            drain(ps, 0)
            ps2 = [pps.tile([128, 512], f32, tag="ps", bufs=2,
                            name=f"vps{i+2}") for i in range(2)]
            for cp in range(4):
                for half in range(2):
                    ct = 2 * cp + half
                    for tc4 in range(2):
                        nc.tensor.matmul(
                            ps2[tc4][:],
                            lhsT=x2s[cp][:, half, (tc4 + 2) * 128:(tc4 + 3) * 128],
                            rhs=wv_s[:, ct, :],
                            start=(ct == 0), stop=(ct == 7))
            drain(ps2, 2)

        def attention_hp(qt, qts, ctxn, hp):
            nki = 4 * qt + 4
            ctx2 = pps.tile([65, 2, 512], f32, tag="ctx2", bufs=1, name="ctx2")
            pending = None   # ctx matmuls delayed one k-tile (SW pipeline)
            for ki in range(nki):
                diag = (ki // 4 == qt)
                off = (ki % 4) * 128 if diag else 0
                S = pps.tile([128, 2, 512], f32, tag="sT2", bufs=2, name="S")
                for hh in range(2):
                    pb = hh * 64
                    ks = kts[ki // 4][pb:pb + 64, hp,
                                      (ki % 4) * 128:(ki % 4 + 1) * 128]
                    qs = qts[pb:pb + 64, hp, off:512]
                    nc.tensor.matmul(S[:, hh, off:], lhsT=ks, rhs=qs,
                                     start=True, stop=True)
                ex = pex.tile([128, 2, 512], bf16, tag="ex", name="ex")
                nc.scalar.activation(ex[:, :, off:], S[:, :, off:], EXP,
                                     scale=0.125)
                if diag:
                    # zero the dead (k>q) triangle of this 128-block for
                    # both heads in one multiply
                    nc.vector.tensor_mul(ex[:, :, off:off + 128],
                                         ex[:, :, off:off + 128], cd2[:])
                cur = (ex, ki, off)
                if pending is not None:
                    pex_t, pki, poff = pending
                    for hh in range(2):
                        h = 2 * hp + hh
                        nc.tensor.matmul(
                            ctx2[0:65, hh, poff:], lhsT=vext[pki][:, h, :],
                            rhs=pex_t[:, hh, poff:],
                            start=(pki == 0), stop=(pki == nki - 1))
                pending = cur
            pex_t, pki, poff = pending
            for hh in range(2):
                h = 2 * hp + hh
                nc.tensor.matmul(
                    ctx2[0:65, hh, poff:], lhsT=vext[pki][:, h, :],
                    rhs=pex_t[:, hh, poff:],
                    start=(pki == 0), stop=(pki == nki - 1))
            for hh in range(2):
                srow = pr.tile([1, 512], f32, tag="srow", name="srow")
                nc.vector.tensor_copy(srow[:], ctx2[64:65, hh, :])
                rrow = pr.tile([1, 512], f32, tag="rrow", name="rrow")
                nc.vector.reciprocal_approx_fast(rrow[:], srow[:])
                rb = pr.tile([64, 512], f32, tag="rb", name="rb")
                nc.gpsimd.partition_broadcast(rb[:], rrow[:])
                if hh == 0:
                    nc.vector.tensor_mul(ctxn[0:64, hp, :],
                                         ctx2[0:64, hh, :], rb[:])
                else:
                    tmp = pr.tile([64, 512], bf16, tag="tmp", name="tmp")
                    nc.vector.tensor_mul(tmp[:], ctx2[0:64, hh, :], rb[:])
                    nc.sync.dma_start(ctxn[64:128, hp, :], tmp[:])

        def outproj_chunk(qt, ctxn, qc4, ch):
            yp = pps.tile([128, 512], f32, tag="ps", bufs=2, name="yp")
            for j in range(4):
                nc.tensor.matmul(
                    yp[:],
                    lhsT=ctxn[:, j, qc4 * 128:(qc4 + 1) * 128],
                    rhs=wo_s[:, j, ch * 512:(ch + 1) * 512],
                    start=(j == 0), stop=(j == 3))
            ys = pys.tile([128, 512], f32, tag="ys", name="ys")
            nc.vector.tensor_copy(ys[:], yp[:])
            nc.sync.dma_start(
                y_d[(qt * 4 + qc4) * 128:(qt * 4 + qc4 + 1) * 128,
                    ch * 512:(ch + 1) * 512], ys[:])


        # Interleave: attention(t4) hp-blocks carry next window's
        # projections and the previous window's out-projection on the
        # PE, keeping it dense (HAM warm) while ScalarE streams exps.
        # warm the ACT exp table during initial DMA
        warm = pr.tile([1, 8], f32, tag="warm", name="warm")
        nc.gpsimd.memset(warm[:], 0.0)
        nc.scalar.activation(warm[:], warm[:], EXP, scale=1.0)
        load_w(wq_s, wq_d)
        qts_cur = proj_qk(wq_s, xq_d, 0, "qts")
        load_w(wk_s, wk_d)
        kts[0] = proj_qk(wk_s, xk_d, 0, "kts")
        load_w(wv_s, wv_d)
        proj_v(0)
        for j in range(4):
            nc.sync.dma_start(wo_s[:, j, :], wo_d[j * 128:(j + 1) * 128, :])
        prev_ctxn = None
        for t4 in range(4):
            ctxn = pctxn.tile([128, 4, 512], bf16, tag="ctxn", name="ctxn")
            qts_next = None
            for hp in range(4):
                attention_hp(t4, qts_cur, ctxn, hp)
                if t4 < 3:
                    if hp == 0:
                        qts_next = proj_qk(wq_s, xq_d, t4 + 1, "qts")
                    elif hp == 1:
                        kts[t4 + 1] = proj_qk(wk_s, xk_d, t4 + 1, "kts")
                    elif hp == 2:
                        proj_v(t4 + 1)
                if prev_ctxn is not None:
                    # spread the previous window's out-projection: 2 of
                    # its 8 chunks after each hp block
                    for c in range(2):
                        idx = hp * 2 + c
                        outproj_chunk(t4 - 1, prev_ctxn, idx // 2, idx % 2)
            prev_ctxn = ctxn
            qts_cur = qts_next
        for qc4 in range(4):
            for ch in range(2):
                outproj_chunk(3, prev_ctxn, qc4, ch)

    nc.compile()
    return nc


def _numpy_fallback(query, key, value, mask, causal_mask, Wq, Wk, Wv, Wo, bo):
    q = (query @ Wq.T).reshape(B, T, H, D).transpose(0, 2, 1, 3)
    k = (key @ Wk.T).reshape(B, T, H, D).transpose(0, 2, 1, 3)
    v = (value @ Wv.T).reshape(B, T, H, D).transpose(0, 2, 1, 3)
    out = np.zeros((B, H, T, D), np.float32)
    for b in range(B):
        for h in range(H):
            s = (q[b, h] @ k[b, h].T) / np.sqrt(np.float32(D))
            s = np.where(mask[b, 0, 0][None, :] == 0, -np.inf, s)
            if causal_mask:
                tri = np.tril(np.ones((T, T), bool))
                s = np.where(tri, s, -np.inf)
            s = s - s.max(axis=-1, keepdims=True)
            e = np.exp(s)
            a = e / e.sum(axis=-1, keepdims=True)
            out[b, h] = a @ v[b, h]
    out = out.transpose(0, 2, 1, 3).reshape(B, T, C)
    return out @ Wo.T + bo


def _in_maps(query, key, value, Wq, Wk, Wv, Wo):
    cdiag = (np.arange(128)[:, None] <= np.arange(128)[None, :]
             ).astype(ml_dtypes.bfloat16)
    cd2 = np.concatenate([cdiag, cdiag], axis=1)   # [128, 256]
    in_maps = []
    for core in range(8):
        b, g = core // 2, core % 2
        hs = g * HD
        in_maps.append({
            "xqT": np.ascontiguousarray(query[b].T).astype(ml_dtypes.bfloat16),
            "xkT": np.ascontiguousarray(key[b].T).astype(ml_dtypes.bfloat16),
            "xvT": np.ascontiguousarray(value[b].T).astype(ml_dtypes.bfloat16),
            "wq": np.ascontiguousarray(Wq[hs:hs + HD, :].T).astype(ml_dtypes.bfloat16),
            "wk": np.ascontiguousarray(Wk[hs:hs + HD, :].T).astype(ml_dtypes.bfloat16),
            "wv": np.ascontiguousarray(Wv[hs:hs + HD, :].T).astype(ml_dtypes.bfloat16),
            "wo": np.ascontiguousarray(Wo[:, hs:hs + HD].T).astype(ml_dtypes.bfloat16),
            "cdiag": cd2,
        })
    return in_maps


def kernel(**inputs):
    from concourse import bass_utils

    inp = {k: np.asarray(v) for k, v in inputs.items()}
    query, key, value = inp["query"], inp["key"], inp["value"]
    Wq, Wk, Wv, Wo, bo = inp["Wq"], inp["Wk"], inp["Wv"], inp["Wo"], inp["bo"]
    mask, causal_mask = inp["mask"], int(inp["causal_mask"])

    if (mask == 0).any() or causal_mask != 1:
        return _numpy_fallback(
            query.astype(np.float32), key.astype(np.float32),
            value.astype(np.float32), mask, causal_mask,
            Wq.astype(np.float32), Wk.astype(np.float32),
            Wv.astype(np.float32), Wo.astype(np.float32),
            bo.astype(np.float32))

    if "nc" not in _CACHE:
        _CACHE["nc"] = _build()
    nc = _CACHE["nc"]

    in_maps = _in_maps(query, key, value, Wq, Wk, Wv, Wo)
    res = bass_utils.run_bass_kernel_spmd(nc, in_maps, core_ids=list(range(8)))
    out = np.zeros((B, T, C), np.float32)
    for core in range(8):
        out[core // 2] += res.results[core]["y"]
    out += bo.astype(np.float32)
    return out


def run_traced(tmpdir=None, **inputs):
    """Profiled run (test harness helper): returns BassKernelResults with
    exec_time_ns/trace populated when the axon NTFF hook is available."""
    from concourse import bass_utils

    inp = {k: np.asarray(v) for k, v in inputs.items()}
    if "nc" not in _CACHE:
        _CACHE["nc"] = _build()
    nc = _CACHE["nc"]
    in_maps = _in_maps(inp["query"], inp["key"], inp["value"],
                       inp["Wq"], inp["Wk"], inp["Wv"], inp["Wo"])
    return bass_utils.run_bass_kernel_spmd(
        nc, in_maps, core_ids=list(range(8)), trace=True, tmpdir=tmpdir)


# revision 3
# speedup vs baseline: 1.0571x; 1.0342x over previous
"""Multi-head attention forward (B=4,T=2048,C=1024,H=16 causal) on 8 TRN2
NeuronCores via Bass/Tile.

Sharding: batch x head-group. Core c handles batch b=c//2 and heads
[g*8,(g+1)*8) where g=c%2. Each core computes its QKV projections
(column-sharded), causal attention for its 8 heads, and a row-sharded
partial of the output projection. The host sums the two partials per
batch and adds the bias.

Device schedule: attention is ACT-paced (exp floor ~853ns/ki-step vs
639ns of PE work), so every projection / out-projection is decomposed
into micro-op closures consumed a few per ki-step inside the attention
loops.  PSUM banks: 4 = score-pair tiles (2x [128,2,512]), 2 = ctx pair,
2 = proj/outproj ping-pong.  The two heads of an hp-pair live in
partitions 0-63 / 64-127 so their K=64 score matmuls row-tile into the
PE array concurrently, and one wide ACTIVATE exponentiates both heads.
"""
import sys
sys.path.insert(0, '/opt/trn_rl_repo')

from collections import deque

import numpy as np
import ml_dtypes

B, T, C, H, D = 4, 2048, 1024, 16, 64
HPC, HD = 8, 512            # heads per core, local head-dim total

_CACHE = {}


def _build():
    from contextlib import ExitStack
    import concourse.bacc as bacc
    import concourse.tile as tile
    from concourse import mybir

    f32 = mybir.dt.float32
    bf16 = mybir.dt.bfloat16
    EXP = mybir.ActivationFunctionType.Exp

    nc = bacc.Bacc("TRN2", target_bir_lowering=False, debug=False, num_devices=1)

    xq_d = nc.dram_tensor("xqT", [C, T], bf16, kind="ExternalInput").ap()
    xk_d = nc.dram_tensor("xkT", [C, T], bf16, kind="ExternalInput").ap()
    xv_d = nc.dram_tensor("xvT", [C, T], bf16, kind="ExternalInput").ap()
    wq_d = nc.dram_tensor("wq", [C, HD], bf16, kind="ExternalInput").ap()
    wk_d = nc.dram_tensor("wk", [C, HD], bf16, kind="ExternalInput").ap()
    wv_d = nc.dram_tensor("wv", [C, HD], bf16, kind="ExternalInput").ap()
    wo_d = nc.dram_tensor("wo", [HD, C], bf16, kind="ExternalInput").ap()
    cd_d = nc.dram_tensor("cdiag", [128, 256], bf16, kind="ExternalInput").ap()
    y_d = nc.dram_tensor("y", [T, C], f32, kind="ExternalOutput").ap()

    with tile.TileContext(nc) as tc, ExitStack() as ctx:
        pw = ctx.enter_context(tc.tile_pool(name="pw", bufs=1))
        pqts = ctx.enter_context(tc.tile_pool(name="pqts", bufs=3))
        pkts = ctx.enter_context(tc.tile_pool(name="pkts", bufs=4))
        pvext = ctx.enter_context(tc.tile_pool(name="pvext", bufs=16))
        pctxn = ctx.enter_context(tc.tile_pool(name="pctxn", bufs=2))
        px = ctx.enter_context(tc.tile_pool(name="px", bufs=8))
        pex = ctx.enter_context(tc.tile_pool(name="pex", bufs=8))
        pr = ctx.enter_context(tc.tile_pool(name="pr", bufs=4))
        pys = ctx.enter_context(tc.tile_pool(name="pys", bufs=3))
        pps = ctx.enter_context(tc.tile_pool(name="pps", bufs=1, space="PSUM"))

        # ---- PE warmup in the (not yet used) score-tile banks: keeps HAM
        # busy during the initial weight/x DMAs without blocking the
        # projection ping-pong banks.
        warmT = pw.tile([128, 128], bf16, tag="warmT")
        nc.gpsimd.memset(warmT[:], 0.0)
        wps = pps.tile([128, 2, 512], f32, tag="sT2", bufs=2, name="wps")
        for _ in range(48):
            nc.tensor.matmul(wps[:, 0, 0:128], lhsT=warmT[:], rhs=warmT[:],
                             start=True, stop=True)

        cd2 = pw.tile([128, 2, 128], bf16, tag="cd2")
        nc.sync.dma_start(cd2[:], cd_d[:].rearrange("p (two q) -> p two q", two=2))
        wq_s = pw.tile([128, 8, HD], bf16, tag="wq")
        wk_s = pw.tile([128, 8, HD], bf16, tag="wk")
        wv_s = pw.tile([128, 8, HD], bf16, tag="wv")
        wo_s = pw.tile([128, 4, C], bf16, tag="wo")

        def load_w(w_s, w_src):
            for ct in range(8):
                nc.sync.dma_start(w_s[:, ct, :],
                                  w_src[ct * 128:(ct + 1) * 128, :])

        kts = [None] * 4     # kT window tiles [128, 4, 512]
        vext = [None] * 16   # vext chunk tiles [128, 8, 65] bf16

        def load_x2(x_src, cp, t4):
            """One 256KB DMA: c-tiles 2cp,2cp+1 of window t4 -> [128,2,512]."""
            x2 = px.tile([128, 2, 512], bf16, tag="x", name="x2")
            nc.sync.dma_start(
                x2[:],
                x_src[2 * cp * 128:(2 * cp + 2) * 128,
                      t4 * 512:(t4 + 1) * 512].rearrange(
                          "(two p) t -> p two t", p=128))
            return x2

        # ---- micro-op streams ------------------------------------------
        # Each returns a list of closures issuing ~one PE instruction
        # (or a cheap DMA/copy).  Groups are strictly sequential so the
        # 2-bank "ps" ping-pong is never over-subscribed.

        def proj_qk_ops(w_s, x_src, t4, tag, holder):
            st = {}
            ops = []

            def mk_dst():
                holder["dst"] = (pqts if tag == "qts" else pkts).tile(
                    [128, 4, 512], bf16, tag=tag, name=tag)
                if tag == "kts":
                    kts[t4] = holder["dst"]

            def load(cp):
                st[cp] = load_x2(x_src, cp, t4)

            def mm(j, ct):
                if ct == 0:
                    st["ps"] = pps.tile([128, 512], f32, tag="ps", bufs=2,
                                        name="ps")
                cp, half = divmod(ct, 2)
                nc.tensor.matmul(
                    st["ps"][:],
                    lhsT=w_s[:, ct, j * 128:(j + 1) * 128],
                    rhs=st[cp][:, half, :],
                    start=(ct == 0), stop=(ct == 7))

            def cp_out(j):
                nc.vector.tensor_copy(holder["dst"][:, j, :], st["ps"][:])

            ops.append(mk_dst)
            for cp in range(4):
                ops.append(lambda cp=cp: load(cp))
            for j in range(4):
                for ct in range(8):
                    ops.append(lambda j=j, ct=ct: mm(j, ct))
                ops.append(lambda j=j: cp_out(j))
            return ops

        def proj_v_ops(t4):
            st = {}
            ops = []

            def load(cp):
                st[cp] = load_x2(xv_d, cp, t4)

            def mm(tc4, ct):
                if ct == 0:
                    st["ps"] = pps.tile([128, 512], f32, tag="ps", bufs=2,
                                        name="vps")
                cp, half = divmod(ct, 2)
                nc.tensor.matmul(
                    st["ps"][:],
                    lhsT=st[cp][:, half, tc4 * 128:(tc4 + 1) * 128],
                    rhs=wv_s[:, ct, :],
                    start=(ct == 0), stop=(ct == 7))

            def vx_out(tc4):
                vx = pvext.tile([128, 8, 65], bf16, tag="vext", name="vx")
                nc.vector.tensor_copy(
                    vx[:, :, 0:64],
                    st["ps"][:].rearrange("p (h d) -> p h d", h=8))
                nc.gpsimd.memset(vx[:, :, 64:65], 1.0)
                vext[4 * t4 + tc4] = vx

            for cp in range(4):
                ops.append(lambda cp=cp: load(cp))
            for tc4 in range(4):
                for ct in range(8):
                    ops.append(lambda tc4=tc4, ct=ct: mm(tc4, ct))
                ops.append(lambda tc4=tc4: vx_out(tc4))
            return ops

        def outproj_ops(qt, ctxn):
            st = {}
            ops = []

            def mm(qc4, ch, j):
                if j == 0:
                    st["yp"] = pps.tile([128, 512], f32, tag="ps", bufs=2,
                                        name="yp")
                nc.tensor.matmul(
                    st["yp"][:],
                    lhsT=ctxn[:, j, qc4 * 128:(qc4 + 1) * 128],
                    rhs=wo_s[:, j, ch * 512:(ch + 1) * 512],
                    start=(j == 0), stop=(j == 3))

            def out(qc4, ch):
                ys = pys.tile([128, 512], f32, tag="ys", name="ys")
                nc.vector.tensor_copy(ys[:], st["yp"][:])
                nc.sync.dma_start(
                    y_d[(qt * 4 + qc4) * 128:(qt * 4 + qc4 + 1) * 128,
                        ch * 512:(ch + 1) * 512], ys[:])

            for qc4 in range(4):
                for ch in range(2):
                    for j in range(4):
                        ops.append(lambda qc4=qc4, ch=ch, j=j: mm(qc4, ch, j))
                    ops.append(lambda qc4=qc4, ch=ch: out(qc4, ch))
            return ops

        # ---- attention -------------------------------------------------
        state = {"steps": 0}   # remaining ki-steps in current window

        def drain(fillers, n):
            for _ in range(n):
                if fillers:
                    fillers.popleft()()

        def attention_hp(qt, qts, ctxn, hp, fillers):
            nki = 4 * qt + 4
            ctx2 = pps.tile([65, 2, 512], f32, tag="ctx2", bufs=1, name="ctx2")
            pending = None
            for ki in range(nki):
                diag = (ki // 4 == qt)
                off = (ki % 4) * 128 if diag else 0
                S = pps.tile([128, 2, 512], f32, tag="sT2", bufs=2, name="S")
                for hh in range(2):
                    pb = hh * 64
                    ks = kts[ki // 4][pb:pb + 64, hp,
                                      (ki % 4) * 128:(ki % 4 + 1) * 128]
                    qs = qts[pb:pb + 64, hp, off:512]
                    nc.tensor.matmul(S[:, hh, off:], lhsT=ks, rhs=qs,
                                     start=True, stop=True)
                ex = pex.tile([128, 2, 512], bf16, tag="ex", name="ex")
                nc.scalar.activation(ex[:, :, off:], S[:, :, off:], EXP,
                                     scale=0.125)
                if diag:
                    nc.vector.tensor_mul(ex[:, :, off:off + 128],
                                         ex[:, :, off:off + 128], cd2[:])
                if pending is not None:
                    pex_t, pki, poff = pending
                    for hh in range(2):
                        h = 2 * hp + hh
                        nc.tensor.matmul(
                            ctx2[0:65, hh, poff:], lhsT=vext[pki][:, h, :],
                            rhs=pex_t[:, hh, poff:],
                            start=(pki == 0), stop=(pki == nki - 1))
                pending = (ex, ki, off)
                n = -(-len(fillers) // state["steps"])   # ceil
                state["steps"] -= 1
                drain(fillers, n)
            pex_t, pki, poff = pending
            for hh in range(2):
                h = 2 * hp + hh
                nc.tensor.matmul(
                    ctx2[0:65, hh, poff:], lhsT=vext[pki][:, h, :],
                    rhs=pex_t[:, hh, poff:],
                    start=(pki == 0), stop=(pki == nki - 1))
            for hh in range(2):
                srow = pr.tile([1, 512], f32, tag="srow", name="srow")
                nc.vector.tensor_copy(srow[:], ctx2[64:65, hh, :])
                rrow = pr.tile([1, 512], f32, tag="rrow", name="rrow")
                nc.vector.reciprocal_approx_fast(rrow[:], srow[:])
                rb = pr.tile([64, 512], f32, tag="rb", name="rb")
                nc.gpsimd.partition_broadcast(rb[:], rrow[:])
                if hh == 0:
                    nc.vector.tensor_mul(ctxn[0:64, hp, :],
                                         ctx2[0:64, hh, :], rb[:])
                else:
                    tmp = pr.tile([64, 512], bf16, tag="tmp", name="tmp")
                    nc.vector.tensor_mul(tmp[:], ctx2[0:64, hh, :], rb[:])
                    nc.sync.dma_start(ctxn[64:128, hp, :], tmp[:])

        # ---- program ---------------------------------------------------
        warm = pr.tile([1, 8], f32, tag="warm", name="warm")
        nc.gpsimd.memset(warm[:], 0.0)
        nc.scalar.activation(warm[:], warm[:], EXP, scale=1.0)

        holders = {("q", t): {} for t in range(4)}
        # preamble: window-0 projections run inline (PE-dense, ACT idle)
        load_w(wq_s, wq_d)
        for op in proj_qk_ops(wq_s, xq_d, 0, "qts", holders[("q", 0)]):
            op()
        load_w(wk_s, wk_d)
        for op in proj_qk_ops(wk_s, xk_d, 0, "kts", {}):
            op()
        load_w(wv_s, wv_d)
        for op in proj_v_ops(0):
            op()
        for j in range(4):
            nc.sync.dma_start(wo_s[:, j, :], wo_d[j * 128:(j + 1) * 128, :])

        prev_ctxn = None
        for t4 in range(4):
            ctxn = pctxn.tile([128, 4, 512], bf16, tag="ctxn", name="ctxn")
            fillers = deque()
            if t4 == 0:
                # window-1 projections
                fillers.extend(proj_qk_ops(wq_s, xq_d, 1, "qts", holders[("q", 1)]))
                fillers.extend(proj_qk_ops(wk_s, xk_d, 1, "kts", {}))
                fillers.extend(proj_v_ops(1))
            elif t4 == 1:
                fillers.extend(proj_qk_ops(wq_s, xq_d, 2, "qts", holders[("q", 2)]))
                fillers.extend(proj_qk_ops(wk_s, xk_d, 2, "kts", {}))
                fillers.extend(proj_v_ops(2))
                fillers.extend(outproj_ops(0, prev_ctxn))
            elif t4 == 2:
                # window-3 q and v projections must finish inside window 2
                # (qts(3) needed at w3 step 0, vext[12..15] at w3 step 12);
                # k(3) is deferred into window 3 itself.
                fillers.extend(proj_qk_ops(wq_s, xq_d, 3, "qts", holders[("q", 3)]))
                fillers.extend(proj_v_ops(3))
                fillers.extend(outproj_ops(1, prev_ctxn))
            else:
                # kts[3] j-group hp is first needed at (3, hp, ki=12);
                # the stream order below meets each deadline.
                k3 = proj_qk_ops(wk_s, xk_d, 3, "kts", {})
                op2 = outproj_ops(2, prev_ctxn)
                fillers.extend(k3[0:14])        # mk_dst, loads, j=0 group
                fillers.extend(op2[0:5])
                fillers.extend(k3[14:23])       # j=1
                fillers.extend(op2[5:10])
                fillers.extend(k3[23:32])       # j=2
                fillers.extend(op2[10:15])
                fillers.extend(k3[32:41])       # j=3
                fillers.extend(op2[15:40])
            state["steps"] = 16 * (t4 + 1)
            qts_cur = holders[("q", t4)]["dst"]
            for hp in range(4):
                attention_hp(t4, qts_cur, ctxn, hp, fillers)
            drain(fillers, len(fillers))
            prev_ctxn = ctxn
        for op in outproj_ops(3, prev_ctxn):
            op()

    nc.compile()
    return nc


def _numpy_fallback(query, key, value, mask, causal_mask, Wq, Wk, Wv, Wo, bo):
    q = (query @ Wq.T).reshape(B, T, H, D).transpose(0, 2, 1, 3)
    k = (key @ Wk.T).reshape(B, T, H, D).transpose(0, 2, 1, 3)
    v = (value @ Wv.T).reshape(B, T, H, D).transpose(0, 2, 1, 3)
    out = np.zeros((B, H, T, D), np.float32)
    for b in range(B):
        for h in range(H):
            s = (q[b, h] @ k[b, h].T) / np.sqrt(np.float32(D))
            s = np.where(mask[b, 0, 0][None, :] == 0, -np.inf, s)
            if causal_mask:
                tri = np.tril(np.ones((T, T), bool))
                s = np.where(tri, s, -np.inf)
            s = s - s.max(axis=-1, keepdims=True)
            e = np.exp(s)
            a = e / e.sum(axis=-1, keepdims=True)
            out[b, h] = a @ v[b, h]
    out = out.transpose(0, 2, 1, 3).reshape(B, T, C)
    return out @ Wo.T + bo


def _in_maps(query, key, value, Wq, Wk, Wv, Wo):
    cdiag = (np.arange(128)[:, None] <= np.arange(128)[None, :]
             ).astype(ml_dtypes.bfloat16)
    cd2 = np.concatenate([cdiag, cdiag], axis=1)   # [128, 256]
    in_maps = []
    for core in range(8):
        b, g = core // 2, core % 2
        hs = g * HD
        in_maps.append({
            "xqT": np.ascontiguousarray(query[b].T).astype(ml_dtypes.bfloat16),
            "xkT": np.ascontiguousarray(key[b].T).astype(ml_dtypes.bfloat16),
            "xvT": np.ascontiguousarray(value[b].T).astype(ml_dtypes.bfloat16),
            "wq": np.ascontiguousarray(Wq[hs:hs + HD, :].T).astype(ml_dtypes.bfloat16),
            "wk": np.ascontiguousarray(Wk[hs:hs + HD, :].T).astype(ml_dtypes.bfloat16),
            "wv": np.ascontiguousarray(Wv[hs:hs + HD, :].T).astype(ml_dtypes.bfloat16),
            "wo": np.ascontiguousarray(Wo[:, hs:hs + HD].T).astype(ml_dtypes.bfloat16),
            "cdiag": cd2,
        })
    return in_maps


def kernel(**inputs):
    from concourse import bass_utils

    inp = {k: np.asarray(v) for k, v in inputs.items()}
    query, key, value = inp["query"], inp["key"], inp["value"]
    Wq, Wk, Wv, Wo, bo = inp["Wq"], inp["Wk"], inp["Wv"], inp["Wo"], inp["bo"]
    mask, causal_mask = inp["mask"], int(inp["causal_mask"])

    if (mask == 0).any() or causal_mask != 1:
        return _numpy_fallback(
            query.astype(np.float32), key.astype(np.float32),
            value.astype(np.float32), mask, causal_mask,
            Wq.astype(np.float32), Wk.astype(np.float32),
            Wv.astype(np.float32), Wo.astype(np.float32),
            bo.astype(np.float32))

    if "nc" not in _CACHE:
        _CACHE["nc"] = _build()
    nc = _CACHE["nc"]

    in_maps = _in_maps(query, key, value, Wq, Wk, Wv, Wo)
    res = bass_utils.run_bass_kernel_spmd(nc, in_maps, core_ids=list(range(8)))
    out = np.zeros((B, T, C), np.float32)
    for core in range(8):
        out[core // 2] += res.results[core]["y"]
    out += bo.astype(np.float32)
    return out


def run_traced(tmpdir=None, **inputs):
    """Profiled run (test harness helper): returns BassKernelResults with
    exec_time_ns/trace populated when the axon NTFF hook is available."""
    from concourse import bass_utils

    inp = {k: np.asarray(v) for k, v in inputs.items()}
    if "nc" not in _CACHE:
        _CACHE["nc"] = _build()
    nc = _CACHE["nc"]
    in_maps = _in_maps(inp["query"], inp["key"], inp["value"],
                       inp["Wq"], inp["Wk"], inp["Wv"], inp["Wo"])
    return bass_utils.run_bass_kernel_spmd(
        nc, in_maps, core_ids=list(range(8)), trace=True, tmpdir=tmpdir)


# revision 10
# speedup vs baseline: 1.0742x; 1.0162x over previous
"""Multi-head attention forward (B=4,T=2048,C=1024,H=16 causal) on 8 TRN2
NeuronCores via Bass/Tile.

Sharding: batch x head-group. Core c handles batch b=c//2 and heads
[g*8,(g+1)*8) where g=c%2. Each core computes its QKV projections
(column-sharded), causal attention for its 8 heads, and a row-sharded
partial of the output projection. The host sums the two partials per
batch and adds the bias.

Device schedule: attention is ACT-paced (exp floor ~853ns/ki-step vs
639ns of PE work), so every projection / out-projection is decomposed
into micro-op closures consumed a few per ki-step inside the attention
loops.  PSUM banks: 4 = score-pair tiles (2x [128,2,512]), 2 = ctx pair,
2 = proj/outproj ping-pong.  The two heads of an hp-pair live in
partitions 0-63 / 64-127 so their K=64 score matmuls row-tile into the
PE array concurrently, and one wide ACTIVATE exponentiates both heads.
"""
import sys
sys.path.insert(0, '/opt/trn_rl_repo')

from collections import deque

import numpy as np
import ml_dtypes

B, T, C, H, D = 4, 2048, 1024, 16, 64
HPC, HD = 8, 512            # heads per core, local head-dim total

_CACHE = {}


def _build():
    from contextlib import ExitStack
    import concourse.bacc as bacc
    import concourse.tile as tile
    from concourse import mybir

    f32 = mybir.dt.float32
    bf16 = mybir.dt.bfloat16
    EXP = mybir.ActivationFunctionType.Exp

    nc = bacc.Bacc("TRN2", target_bir_lowering=False, debug=False, num_devices=1)

    xq_d = nc.dram_tensor("xqT", [C, T], bf16, kind="ExternalInput").ap()
    xk_d = nc.dram_tensor("xkT", [C, T], bf16, kind="ExternalInput").ap()
    xv_d = nc.dram_tensor("xvT", [C, T], bf16, kind="ExternalInput").ap()
    wq_d = nc.dram_tensor("wq", [C, HD], bf16, kind="ExternalInput").ap()
    wk_d = nc.dram_tensor("wk", [C, HD], bf16, kind="ExternalInput").ap()
    wv_d = nc.dram_tensor("wv", [C, HD], bf16, kind="ExternalInput").ap()
    wo_d = nc.dram_tensor("wo", [HD, C], bf16, kind="ExternalInput").ap()
    cd_d = nc.dram_tensor("cdiag", [128, 256], bf16, kind="ExternalInput").ap()
    y_d = nc.dram_tensor("y", [T, C], bf16, kind="ExternalOutput").ap()

    with tile.TileContext(nc) as tc, ExitStack() as ctx:
        pw = ctx.enter_context(tc.tile_pool(name="pw", bufs=1))
        pqts = ctx.enter_context(tc.tile_pool(name="pqts", bufs=3))
        pkts = ctx.enter_context(tc.tile_pool(name="pkts", bufs=4))
        pvext = ctx.enter_context(tc.tile_pool(name="pvext", bufs=16))
        pctxn = ctx.enter_context(tc.tile_pool(name="pctxn", bufs=2))
        px = ctx.enter_context(tc.tile_pool(name="px", bufs=8))
        pex = ctx.enter_context(tc.tile_pool(name="pex", bufs=10))
        pr = ctx.enter_context(tc.tile_pool(name="pr", bufs=4))
        pys = ctx.enter_context(tc.tile_pool(name="pys", bufs=3))
        pps = ctx.enter_context(tc.tile_pool(name="pps", bufs=1, space="PSUM"))

        # ---- PE warmup in the (not yet used) score-tile banks: keeps HAM
        # busy during the initial weight/x DMAs without blocking the
        # projection ping-pong banks.
        warmT = pw.tile([128, 128], bf16, tag="warmT")
        nc.gpsimd.memset(warmT[:], 0.0)
        wps = pps.tile([128, 2, 512], f32, tag="sT2", bufs=2, name="wps")
        for _ in range(48):
            nc.tensor.matmul(wps[:, 0, 0:128], lhsT=warmT[:], rhs=warmT[:],
                             start=True, stop=True)

        cd2 = pw.tile([128, 2, 128], bf16, tag="cd2")
        nc.sync.dma_start(cd2[:], cd_d[:].rearrange("p (two q) -> p two q", two=2))
        wq_s = pw.tile([128, 8, HD], bf16, tag="wq")
        wk_s = pw.tile([128, 8, HD], bf16, tag="wk")
        wv_s = pw.tile([128, 8, HD], bf16, tag="wv")
        wo_s = pw.tile([128, 4, C], bf16, tag="wo")

        def load_w(w_s, w_src):
            for ct in range(8):
                nc.sync.dma_start(w_s[:, ct, :],
                                  w_src[ct * 128:(ct + 1) * 128, :])

        kts = [None] * 4     # kT window tiles [128, 4, 512]
        vext = [None] * 16   # vext chunk tiles [128, 8, 65] bf16

        def load_x2(x_src, cp, t4):
            """One 256KB DMA: c-tiles 2cp,2cp+1 of window t4 -> [128,2,512]."""
            x2 = px.tile([128, 2, 512], bf16, tag="x", name="x2")
            nc.sync.dma_start(
                x2[:],
                x_src[2 * cp * 128:(2 * cp + 2) * 128,
                      t4 * 512:(t4 + 1) * 512].rearrange(
                          "(two p) t -> p two t", p=128))
            return x2

        # ---- micro-op streams ------------------------------------------
        # Each returns a list of closures issuing ~one PE instruction
        # (or a cheap DMA/copy).  Groups are strictly sequential so the
        # 2-bank "ps" ping-pong is never over-subscribed.

        def proj_qk_ops(w_s, x_src, t4, tag, holder):
            st = {}
            ops = []

            def mk_dst():
                holder["dst"] = (pqts if tag == "qts" else pkts).tile(
                    [128, 4, 512], bf16, tag=tag, name=tag)
                if tag == "kts":
                    kts[t4] = holder["dst"]

            def load(cp):
                st[cp] = load_x2(x_src, cp, t4)

            def mm(j, ct):
                if ct == 0:
                    st["ps"] = pps.tile([128, 512], f32, tag="ps", bufs=2,
                                        name="ps")
                cp, half = divmod(ct, 2)
                nc.tensor.matmul(
                    st["ps"][:],
                    lhsT=w_s[:, ct, j * 128:(j + 1) * 128],
                    rhs=st[cp][:, half, :],
                    start=(ct == 0), stop=(ct == 7))

            def cp_out(j):
                nc.vector.tensor_copy(holder["dst"][:, j, :], st["ps"][:])

            ops.append(mk_dst)
            for cp in range(4):
                ops.append(lambda cp=cp: load(cp))
            for j in range(4):
                for ct in range(8):
                    ops.append(lambda j=j, ct=ct: mm(j, ct))
                ops.append(lambda j=j: cp_out(j))
            return ops

        def proj_v_ops(t4):
            st = {}
            ops = []

            def load(cp):
                st[cp] = load_x2(xv_d, cp, t4)

            def mm(tc4, ct):
                if ct == 0:
                    st["ps"] = pps.tile([128, 512], f32, tag="ps", bufs=2,
                                        name="vps")
                cp, half = divmod(ct, 2)
                nc.tensor.matmul(
                    st["ps"][:],
                    lhsT=st[cp][:, half, tc4 * 128:(tc4 + 1) * 128],
                    rhs=wv_s[:, ct, :],
                    start=(ct == 0), stop=(ct == 7))

            def vx_out(tc4):
                vx = pvext.tile([128, 8, 65], bf16, tag="vext", name="vx")
                nc.vector.tensor_copy(
                    vx[:, :, 0:64],
                    st["ps"][:].rearrange("p (h d) -> p h d", h=8))
                nc.gpsimd.memset(vx[:, :, 64:65], 1.0)
                vext[4 * t4 + tc4] = vx

            for cp in range(4):
                ops.append(lambda cp=cp: load(cp))
            for tc4 in range(4):
                for ct in range(8):
                    ops.append(lambda tc4=tc4, ct=ct: mm(tc4, ct))
                ops.append(lambda tc4=tc4: vx_out(tc4))
            return ops

        def outproj_ops(qt, ctxn):
            st = {}
            ops = []

            def mm(qc4, ch, j):
                if j == 0:
                    st["yp"] = pps.tile([128, 512], f32, tag="ps", bufs=2,
                                        name="yp")
                nc.tensor.matmul(
                    st["yp"][:],
                    lhsT=ctxn[:, j, qc4 * 128:(qc4 + 1) * 128],
                    rhs=wo_s[:, j, ch * 512:(ch + 1) * 512],
                    start=(j == 0), stop=(j == 3))

            def out(qc4, ch):
                ys = pys.tile([128, 512], bf16, tag="ys", name="ys")
                nc.vector.tensor_copy(ys[:], st["yp"][:])
                nc.sync.dma_start(
                    y_d[(qt * 4 + qc4) * 128:(qt * 4 + qc4 + 1) * 128,
                        ch * 512:(ch + 1) * 512], ys[:])

            for qc4 in range(4):
                for ch in range(2):
                    for j in range(4):
                        ops.append(lambda qc4=qc4, ch=ch, j=j: mm(qc4, ch, j))
                    ops.append(lambda qc4=qc4, ch=ch: out(qc4, ch))
            return ops

        def outproj_tail(qt, ctxn):
            """Final window's out-projection: attention PSUM is dead, so
            borrow the score/ctx banks — 8 independent banks, zero
            rotation stalls."""
            tA = pps.tile([128, 2, 512], f32, tag="sT2", bufs=2, name="tA")
            tB = pps.tile([128, 2, 512], f32, tag="sT2", bufs=2, name="tB")
            tC = pps.tile([128, 2, 512], f32, tag="ctx2", bufs=1, name="tC")
            slots = [tA[:, 0, :], tA[:, 1, :], tB[:, 0, :], tB[:, 1, :],
                     tC[:, 0, :], tC[:, 1, :]]
            idx = 0
            for qc4 in range(4):
                for ch in range(2):
                    if idx < 6:
                        yp = slots[idx]
                    else:
                        yp = pps.tile([128, 512], f32, tag="ps", bufs=2,
                                      name="yp")[:]
                    idx += 1
                    for j in range(4):
                        nc.tensor.matmul(
                            yp,
                            lhsT=ctxn[:, j, qc4 * 128:(qc4 + 1) * 128],
                            rhs=wo_s[:, j, ch * 512:(ch + 1) * 512],
                            start=(j == 0), stop=(j == 3))
                    ys = pys.tile([128, 512], bf16, tag="ys", name="ys")
                    nc.vector.tensor_copy(ys[:], yp)
                    nc.sync.dma_start(
                        y_d[(qt * 4 + qc4) * 128:(qt * 4 + qc4 + 1) * 128,
                            ch * 512:(ch + 1) * 512], ys[:])

        # ---- attention -------------------------------------------------
        state = {"steps": 0}   # remaining ki-steps in current window

        def drain(fillers, n):
            for _ in range(n):
                if fillers:
                    fillers.popleft()()

        def attention_hp(qt, qts, ctxn, hp, fillers):
            nki = 4 * qt + 4
            ctx2 = pps.tile([65, 2, 512], f32, tag="ctx2", bufs=1, name="ctx2")

            def attnv(item):
                pex_t, pki, poff = item
                for hh in range(2):
                    h = 2 * hp + hh
                    nc.tensor.matmul(
                        ctx2[0:65, hh, poff:], lhsT=vext[pki][:, h, :],
                        rhs=pex_t[:, hh, poff:],
                        start=(pki == 0), stop=(pki == nki - 1))

            # depth-2 SW pipeline: attnV(ki) issues at ki+2 so the
            # previous hp's normalization never heads the in-order PE
            # queue while scores/exps are ready behind it.
            pending = deque()
            for ki in range(nki):
                diag = (ki // 4 == qt)
                off = (ki % 4) * 128 if diag else 0
                S = pps.tile([128, 2, 512], f32, tag="sT2", bufs=2, name="S")
                for hh in range(2):
                    pb = hh * 64
                    ks = kts[ki // 4][pb:pb + 64, hp,
                                      (ki % 4) * 128:(ki % 4 + 1) * 128]
                    qs = qts[pb:pb + 64, hp, off:512]
                    nc.tensor.matmul(S[:, hh, off:], lhsT=ks, rhs=qs,
                                     start=True, stop=True)
                ex = pex.tile([128, 2, 512], bf16, tag="ex", name="ex")
                nc.scalar.activation(ex[:, :, off:], S[:, :, off:], EXP,
                                     scale=0.125)
                if diag:
                    nc.vector.tensor_mul(ex[:, :, off:off + 128],
                                         ex[:, :, off:off + 128], cd2[:])
                pending.append((ex, ki, off))
                if len(pending) > 2:
                    attnv(pending.popleft())
                n = -(-len(fillers) // state["steps"])   # ceil
                state["steps"] -= 1
                drain(fillers, n)
            while pending:
                attnv(pending.popleft())
            for hh in range(2):
                srow = pr.tile([1, 512], f32, tag="srow", name="srow")
                nc.vector.tensor_copy(srow[:], ctx2[64:65, hh, :])
                rrow = pr.tile([1, 512], f32, tag="rrow", name="rrow")
                nc.vector.reciprocal_approx_fast(rrow[:], srow[:])
                rb = pr.tile([64, 512], f32, tag="rb", name="rb")
                nc.gpsimd.partition_broadcast(rb[:], rrow[:])
                if hh == 0:
                    nc.vector.tensor_mul(ctxn[0:64, hp, :],
                                         ctx2[0:64, hh, :], rb[:])
                else:
                    tmp = pr.tile([64, 512], bf16, tag="tmp", name="tmp")
                    nc.vector.tensor_mul(tmp[:], ctx2[0:64, hh, :], rb[:])
                    nc.sync.dma_start(ctxn[64:128, hp, :], tmp[:])

        # ---- program ---------------------------------------------------
        warm = pr.tile([1, 8], f32, tag="warm", name="warm")
        nc.gpsimd.memset(warm[:], 0.0)
        nc.scalar.activation(warm[:], warm[:], EXP, scale=1.0)

        holders = {("q", t): {} for t in range(4)}
        # preamble: window-0 projections run inline (PE-dense, ACT idle)
        load_w(wq_s, wq_d)
        for op in proj_qk_ops(wq_s, xq_d, 0, "qts", holders[("q", 0)]):
            op()
        load_w(wk_s, wk_d)
        for op in proj_qk_ops(wk_s, xk_d, 0, "kts", {}):
            op()
        load_w(wv_s, wv_d)
        for op in proj_v_ops(0):
            op()
        for j in range(4):
            nc.sync.dma_start(wo_s[:, j, :], wo_d[j * 128:(j + 1) * 128, :])

        prev_ctxn = None
        for t4 in range(4):
            ctxn = pctxn.tile([128, 4, 512], bf16, tag="ctxn", name="ctxn")
            fillers = deque()
            if t4 == 0:
                # window-1 projections
                fillers.extend(proj_qk_ops(wq_s, xq_d, 1, "qts", holders[("q", 1)]))
                fillers.extend(proj_qk_ops(wk_s, xk_d, 1, "kts", {}))
                fillers.extend(proj_v_ops(1))
            elif t4 == 1:
                fillers.extend(proj_qk_ops(wq_s, xq_d, 2, "qts", holders[("q", 2)]))
                fillers.extend(proj_qk_ops(wk_s, xk_d, 2, "kts", {}))
                fillers.extend(proj_v_ops(2))
                fillers.extend(outproj_ops(0, prev_ctxn))
            elif t4 == 2:
                # window-3 q and v projections must finish inside window 2
                # (qts(3) needed at w3 step 0, vext[12..15] at w3 step 12);
                # k(3) is deferred into window 3 itself.
                fillers.extend(proj_qk_ops(wq_s, xq_d, 3, "qts", holders[("q", 3)]))
                fillers.extend(proj_v_ops(3))
                fillers.extend(outproj_ops(1, prev_ctxn))
            else:
                # kts[3] j-group hp is first needed at (3, hp, ki=12);
                # the stream order below meets each deadline.
                k3 = proj_qk_ops(wk_s, xk_d, 3, "kts", {})
                op2 = outproj_ops(2, prev_ctxn)
                fillers.extend(k3[0:14])        # mk_dst, loads, j=0 group
                fillers.extend(op2[0:5])
                fillers.extend(k3[14:23])       # j=1
                fillers.extend(op2[5:10])
                fillers.extend(k3[23:32])       # j=2
                fillers.extend(op2[10:15])
                fillers.extend(k3[32:41])       # j=3
                fillers.extend(op2[15:40])
            state["steps"] = 16 * (t4 + 1)
            qts_cur = holders[("q", t4)]["dst"]
            for hp in range(4):
                attention_hp(t4, qts_cur, ctxn, hp, fillers)
            drain(fillers, len(fillers))
            prev_ctxn = ctxn
        outproj_tail(3, prev_ctxn)

    nc.compile()
    return nc


def _numpy_fallback(query, key, value, mask, causal_mask, Wq, Wk, Wv, Wo, bo):
    q = (query @ Wq.T).reshape(B, T, H, D).transpose(0, 2, 1, 3)
    k = (key @ Wk.T).reshape(B, T, H, D).transpose(0, 2, 1, 3)
    v = (value @ Wv.T).reshape(B, T, H, D).transpose(0, 2, 1, 3)
    out = np.zeros((B, H, T, D), np.float32)
    for b in range(B):
        for h in range(H):
            s = (q[b, h] @ k[b, h].T) / np.sqrt(np.float32(D))
            s = np.where(mask[b, 0, 0][None, :] == 0, -np.inf, s)
            if causal_mask:
                tri = np.tril(np.ones((T, T), bool))
                s = np.where(tri, s, -np.inf)
            s = s - s.max(axis=-1, keepdims=True)
            e = np.exp(s)
            a = e / e.sum(axis=-1, keepdims=True)
            out[b, h] = a @ v[b, h]
    out = out.transpose(0, 2, 1, 3).reshape(B, T, C)
    return out @ Wo.T + bo


def _in_maps(query, key, value, Wq, Wk, Wv, Wo):
    cdiag = (np.arange(128)[:, None] <= np.arange(128)[None, :]
             ).astype(ml_dtypes.bfloat16)
    cd2 = np.concatenate([cdiag, cdiag], axis=1)   # [128, 256]
    in_maps = []
    for core in range(8):
        b, g = core // 2, core % 2
        hs = g * HD
        in_maps.append({
            "xqT": np.ascontiguousarray(query[b].T).astype(ml_dtypes.bfloat16),
            "xkT": np.ascontiguousarray(key[b].T).astype(ml_dtypes.bfloat16),
            "xvT": np.ascontiguousarray(value[b].T).astype(ml_dtypes.bfloat16),
            "wq": np.ascontiguousarray(Wq[hs:hs + HD, :].T).astype(ml_dtypes.bfloat16),
            "wk": np.ascontiguousarray(Wk[hs:hs + HD, :].T).astype(ml_dtypes.bfloat16),
            "wv": np.ascontiguousarray(Wv[hs:hs + HD, :].T).astype(ml_dtypes.bfloat16),
            "wo": np.ascontiguousarray(Wo[:, hs:hs + HD].T).astype(ml_dtypes.bfloat16),
            "cdiag": cd2,
        })
    return in_maps


def kernel(**inputs):
    from concourse import bass_utils

    inp = {k: np.asarray(v) for k, v in inputs.items()}
    query, key, value = inp["query"], inp["key"], inp["value"]
    Wq, Wk, Wv, Wo, bo = inp["Wq"], inp["Wk"], inp["Wv"], inp["Wo"], inp["bo"]
    mask, causal_mask = inp["mask"], int(inp["causal_mask"])

    if (mask == 0).any() or causal_mask != 1:
        return _numpy_fallback(
            query.astype(np.float32), key.astype(np.float32),
            value.astype(np.float32), mask, causal_mask,
            Wq.astype(np.float32), Wk.astype(np.float32),
            Wv.astype(np.float32), Wo.astype(np.float32),
            bo.astype(np.float32))

    if "nc" not in _CACHE:
        _CACHE["nc"] = _build()
    nc = _CACHE["nc"]

    in_maps = _in_maps(query, key, value, Wq, Wk, Wv, Wo)
    res = bass_utils.run_bass_kernel_spmd(nc, in_maps, core_ids=list(range(8)))
    out = np.zeros((B, T, C), np.float32)
    for core in range(8):
        out[core // 2] += np.asarray(res.results[core]["y"], np.float32)
    out += bo.astype(np.float32)
    return out


def run_traced(tmpdir=None, **inputs):
    """Profiled run (test harness helper): returns BassKernelResults with
    exec_time_ns/trace populated when the axon NTFF hook is available."""
    from concourse import bass_utils

    inp = {k: np.asarray(v) for k, v in inputs.items()}
    if "nc" not in _CACHE:
        _CACHE["nc"] = _build()
    nc = _CACHE["nc"]
    in_maps = _in_maps(inp["query"], inp["key"], inp["value"],
                       inp["Wq"], inp["Wk"], inp["Wv"], inp["Wo"])
    return bass_utils.run_bass_kernel_spmd(
        nc, in_maps, core_ids=list(range(8)), trace=True, tmpdir=tmpdir)


# revision 20
# speedup vs baseline: 1.0783x; 1.0038x over previous
"""Multi-head attention forward (B=4,T=2048,C=1024,H=16 causal) on 8 TRN2
NeuronCores via Bass/Tile.

Sharding: batch x head-group. Core c handles batch b=c//2 and heads
[g*8,(g+1)*8) where g=c%2. Each core computes its QKV projections
(column-sharded), causal attention for its 8 heads, and a row-sharded
partial of the output projection. The host sums the two partials per
batch and adds the bias.

Device schedule: attention is ACT-paced (exp floor ~853ns/ki-step vs
639ns of PE work), so every projection / out-projection is decomposed
into micro-op closures consumed a few per ki-step inside the attention
loops.  PSUM banks: 4 = score-pair tiles (2x [128,2,512]), 2 = ctx pair,
2 = proj/outproj ping-pong.  The two heads of an hp-pair live in
partitions 0-63 / 64-127 so their K=64 score matmuls row-tile into the
PE array concurrently, and one wide ACTIVATE exponentiates both heads.
"""
import sys
sys.path.insert(0, '/opt/trn_rl_repo')

from collections import deque

import numpy as np
import ml_dtypes

B, T, C, H, D = 4, 2048, 1024, 16, 64
HPC, HD = 8, 512            # heads per core, local head-dim total

_CACHE = {}


def _build():
    from contextlib import ExitStack
    import concourse.bacc as bacc
    import concourse.tile as tile
    from concourse import mybir

    f32 = mybir.dt.float32
    bf16 = mybir.dt.bfloat16
    EXP = mybir.ActivationFunctionType.Exp

    nc = bacc.Bacc("TRN2", target_bir_lowering=False, debug=False, num_devices=1)

    xq_d = nc.dram_tensor("xqT", [C, T], bf16, kind="ExternalInput").ap()
    xk_d = nc.dram_tensor("xkT", [C, T], bf16, kind="ExternalInput").ap()
    xv_d = nc.dram_tensor("xvT", [C, T], bf16, kind="ExternalInput").ap()
    wq_d = nc.dram_tensor("wq", [C, HD], bf16, kind="ExternalInput").ap()
    wk_d = nc.dram_tensor("wk", [C, HD], bf16, kind="ExternalInput").ap()
    wv_d = nc.dram_tensor("wv", [C, HD], bf16, kind="ExternalInput").ap()
    wo_d = nc.dram_tensor("wo", [HD, C], bf16, kind="ExternalInput").ap()
    cd_d = nc.dram_tensor("cdiag", [128, 256], bf16, kind="ExternalInput").ap()
    y_d = nc.dram_tensor("y", [T, C], bf16, kind="ExternalOutput").ap()

    with tile.TileContext(nc) as tc, ExitStack() as ctx:
        pw = ctx.enter_context(tc.tile_pool(name="pw", bufs=1))
        pqts = ctx.enter_context(tc.tile_pool(name="pqts", bufs=3))
        pkts = ctx.enter_context(tc.tile_pool(name="pkts", bufs=4))
        pvext = ctx.enter_context(tc.tile_pool(name="pvext", bufs=16))
        pctxn = ctx.enter_context(tc.tile_pool(name="pctxn", bufs=2))
        px = ctx.enter_context(tc.tile_pool(name="px", bufs=8))
        pex = ctx.enter_context(tc.tile_pool(name="pex", bufs=10))
        pr = ctx.enter_context(tc.tile_pool(name="pr", bufs=4))
        pys = ctx.enter_context(tc.tile_pool(name="pys", bufs=3))
        pps = ctx.enter_context(tc.tile_pool(name="pps", bufs=1, space="PSUM"))

        # ---- PE warmup in the (not yet used) score-tile banks: keeps HAM
        # busy during the initial weight/x DMAs without blocking the
        # projection ping-pong banks.
        warmT = pw.tile([128, 128], bf16, tag="warmT")
        nc.gpsimd.memset(warmT[:], 0.0)
        wps = pps.tile([128, 2, 512], f32, tag="sT2", bufs=2, name="wps")
        for _ in range(48):
            nc.tensor.matmul(wps[:, 0, 0:128], lhsT=warmT[:], rhs=warmT[:],
                             start=True, stop=True)

        cd2 = pw.tile([128, 2, 128], bf16, tag="cd2")
        nc.sync.dma_start(cd2[:], cd_d[:].rearrange("p (two q) -> p two q", two=2))
        wq_s = pw.tile([128, 8, HD], bf16, tag="wq")
        wk_s = pw.tile([128, 8, HD], bf16, tag="wk")
        wv_s = pw.tile([128, 8, HD], bf16, tag="wv")
        wo_s = pw.tile([128, 4, C], bf16, tag="wo")

        def load_w(w_s, w_src):
            for ct in range(8):
                nc.sync.dma_start(w_s[:, ct, :],
                                  w_src[ct * 128:(ct + 1) * 128, :])

        kts = [None] * 4     # kT window tiles [128, 4, 512]
        vext = [None] * 16   # vext chunk tiles [128, 8, 65] bf16

        def load_x2(x_src, cp, t4):
            """One 256KB DMA: c-tiles 2cp,2cp+1 of window t4 -> [128,2,512]."""
            x2 = px.tile([128, 2, 512], bf16, tag="x", name="x2")
            nc.sync.dma_start(
                x2[:],
                x_src[2 * cp * 128:(2 * cp + 2) * 128,
                      t4 * 512:(t4 + 1) * 512].rearrange(
                          "(two p) t -> p two t", p=128))
            return x2

        # ---- micro-op streams ------------------------------------------
        # Each projection is split into a cheap "head" (dst alloc + the 4
        # x2 DMA issues) and a "body" of 4 matmul groups.  Heads are
        # scheduled >=1 group ahead of their body so the in-order PE
        # queue never stalls on a just-issued DMA.  Groups are strictly
        # sequential so the 2-bank "ps" ping-pong is never
        # over-subscribed.

        def proj_qk_ops(w_s, x_src, t4, tag, holder):
            st = {}

            def mk_dst():
                holder["dst"] = (pqts if tag == "qts" else pkts).tile(
                    [128, 4, 512], bf16, tag=tag, name=tag)
                if tag == "kts":
                    kts[t4] = holder["dst"]

            def load(cp):
                st[cp] = load_x2(x_src, cp, t4)

            def mm(j, ct):
                if ct == 0:
                    st["ps"] = pps.tile([128, 512], f32, tag="ps", bufs=2,
                                        name="ps")
                cp, half = divmod(ct, 2)
                nc.tensor.matmul(
                    st["ps"][:],
                    lhsT=w_s[:, ct, j * 128:(j + 1) * 128],
                    rhs=st[cp][:, half, :],
                    start=(ct == 0), stop=(ct == 7))

            def cp_out(j):
                nc.vector.tensor_copy(holder["dst"][:, j, :], st["ps"][:])

            head = [mk_dst] + [lambda cp=cp: load(cp) for cp in range(4)]
            groups = []
            for j in range(4):
                g = [lambda j=j, ct=ct: mm(j, ct) for ct in range(8)]
                g.append(lambda j=j: cp_out(j))
                groups.append(g)
            return head, groups

        def proj_v_ops(t4):
            st = {}

            def load(cp):
                st[cp] = load_x2(xv_d, cp, t4)

            def mm(tc4, ct):
                if ct == 0:
                    st["ps"] = pps.tile([128, 512], f32, tag="ps", bufs=2,
                                        name="vps")
                cp, half = divmod(ct, 2)
                nc.tensor.matmul(
                    st["ps"][:],
                    lhsT=st[cp][:, half, tc4 * 128:(tc4 + 1) * 128],
                    rhs=wv_s[:, ct, :],
                    start=(ct == 0), stop=(ct == 7))

            def vx_out(tc4):
                vx = pvext.tile([128, 8, 65], bf16, tag="vext", name="vx")
                nc.vector.tensor_copy(
                    vx[:, :, 0:64],
                    st["ps"][:].rearrange("p (h d) -> p h d", h=8))
                nc.gpsimd.memset(vx[:, :, 64:65], 1.0)
                vext[4 * t4 + tc4] = vx

            head = [lambda cp=cp: load(cp) for cp in range(4)]
            groups = []
            for tc4 in range(4):
                g = [lambda tc4=tc4, ct=ct: mm(tc4, ct) for ct in range(8)]
                g.append(lambda tc4=tc4: vx_out(tc4))
                groups.append(g)
            return head, groups

        def outproj_ops(qt, ctxn):
            st = {}
            ops = []

            def mm(qc4, ch, j):
                if j == 0:
                    st["yp"] = pps.tile([128, 512], f32, tag="ps", bufs=2,
                                        name="yp")
                nc.tensor.matmul(
                    st["yp"][:],
                    lhsT=ctxn[:, j, qc4 * 128:(qc4 + 1) * 128],
                    rhs=wo_s[:, j, ch * 512:(ch + 1) * 512],
                    start=(j == 0), stop=(j == 3))

            def out(qc4, ch):
                ys = pys.tile([128, 512], bf16, tag="ys", name="ys")
                nc.vector.tensor_copy(ys[:], st["yp"][:])
                nc.sync.dma_start(
                    y_d[(qt * 4 + qc4) * 128:(qt * 4 + qc4 + 1) * 128,
                        ch * 512:(ch + 1) * 512], ys[:])

            for qc4 in range(4):
                for ch in range(2):
                    for j in range(4):
                        ops.append(lambda qc4=qc4, ch=ch, j=j: mm(qc4, ch, j))
                    ops.append(lambda qc4=qc4, ch=ch: out(qc4, ch))
            return ops

        def outproj_tail(qt, ctxn):
            """Final window's out-projection: attention PSUM is dead, so
            borrow the score/ctx banks — 8 independent banks, zero
            rotation stalls."""
            tA = pps.tile([128, 2, 512], f32, tag="sT2", bufs=2, name="tA")
            tB = pps.tile([128, 2, 512], f32, tag="sT2", bufs=2, name="tB")
            tC = pps.tile([128, 2, 512], f32, tag="ctx2", bufs=1, name="tC")
            slots = [tA[:, 0, :], tA[:, 1, :], tB[:, 0, :], tB[:, 1, :],
                     tC[:, 0, :], tC[:, 1, :]]
            idx = 0
            for qc4 in range(4):
                for ch in range(2):
                    if idx < 6:
                        yp = slots[idx]
                    else:
                        yp = pps.tile([128, 512], f32, tag="ps", bufs=2,
                                      name="yp")[:]
                    idx += 1
                    for j in range(4):
                        nc.tensor.matmul(
                            yp,
                            lhsT=ctxn[:, j, qc4 * 128:(qc4 + 1) * 128],
                            rhs=wo_s[:, j, ch * 512:(ch + 1) * 512],
                            start=(j == 0), stop=(j == 3))
                    ys = pys.tile([128, 512], bf16, tag="ys", name="ys")
                    nc.vector.tensor_copy(ys[:], yp)
                    nc.sync.dma_start(
                        y_d[(qt * 4 + qc4) * 128:(qt * 4 + qc4 + 1) * 128,
                            ch * 512:(ch + 1) * 512], ys[:])

        # ---- attention -------------------------------------------------
        state = {"steps": 0}   # remaining ki-steps in current window
        pending = deque()      # global depth-2 attnV pipeline, carries
                               # (ex, ki, off, hp, ctx2, nki, ctxn)

        def drain(fillers, n):
            for _ in range(n):
                if fillers:
                    fillers.popleft()()

        def norm_block(ctx2, ctxn, hp):
            """Drain ctx PSUM to SBUF with one copy (frees the banks
            fast), then normalize off the critical path."""
            for hh in range(2):
                srow = pr.tile([1, 512], f32, tag="rrow", name="srow")
                nc.vector.tensor_copy(srow[:], ctx2[64:65, hh, :])
                rrow = pr.tile([1, 512], f32, tag="rrow", name="rrow")
                nc.vector.reciprocal_approx_fast(rrow[:], srow[:])
                rb = pr.tile([64, 512], f32, tag="rb", name="rb")
                nc.gpsimd.partition_broadcast(rb[:], rrow[:])
                if hh == 0:
                    nc.vector.tensor_mul(ctxn[0:64, hp, :],
                                         ctx2[0:64, hh, :], rb[:])
                else:
                    tmp = pr.tile([64, 512], bf16, tag="tmp", name="tmp")
                    nc.vector.tensor_mul(tmp[:], ctx2[0:64, hh, :], rb[:])
                    nc.sync.dma_start(ctxn[64:128, hp, :], tmp[:])

        def attnv_pop():
            ex, pki, poff, hp, blk, nki, ctxn = pending.popleft()
            if pki == 0:
                # allocate lazily at the first write: by now the previous
                # block's drain copy has been issued, so the bank-rotation
                # guard orders this block's writes after it.
                blk["ctx2"] = pps.tile([65, 2, 512], f32, tag="ctx2",
                                       bufs=1, name="ctx2")
            ctx2 = blk["ctx2"]
            for hh in range(2):
                h = 2 * hp + hh
                nc.tensor.matmul(
                    ctx2[0:65, hh, poff:], lhsT=vext[pki][:, h, :],
                    rhs=ex[:, hh, poff:],
                    start=(pki == 0), stop=(pki == nki - 1))
            if pki == nki - 1:
                norm_block(ctx2, ctxn, hp)

        def attention_hp(qt, qts, ctxn, hp, fillers):
            nki = 4 * qt + 4
            blk = {}
            for ki in range(nki):
                diag = (ki // 4 == qt)
                off = (ki % 4) * 128 if diag else 0
                S = pps.tile([128, 2, 512], f32, tag="sT2", bufs=2, name="S")
                for hh in range(2):
                    pb = hh * 64
                    ks = kts[ki // 4][pb:pb + 64, hp,
                                      (ki % 4) * 128:(ki % 4 + 1) * 128]
                    qs = qts[pb:pb + 64, hp, off:512]
                    nc.tensor.matmul(S[:, hh, off:], lhsT=ks, rhs=qs,
                                     start=True, stop=True)
                ex = pex.tile([128, 2, 512], bf16, tag="ex", name="ex")
                nc.scalar.activation(ex[:, :, off:], S[:, :, off:], EXP,
                                     scale=0.125)
                if diag:
                    nc.vector.tensor_mul(ex[:, :, off:off + 128],
                                         ex[:, :, off:off + 128], cd2[:])
                pending.append((ex, ki, off, hp, blk, nki, ctxn))
                if len(pending) > 2:
                    attnv_pop()
                n = -(-len(fillers) // state["steps"])   # ceil
                state["steps"] -= 1
                drain(fillers, n)
            while pending:   # BISECT: per-block flush
                attnv_pop()

        # ---- program ---------------------------------------------------
        warm = pr.tile([1, 8], f32, tag="warm", name="warm")
        nc.gpsimd.memset(warm[:], 0.0)
        nc.scalar.activation(warm[:], warm[:], EXP, scale=1.0)

        holders = {("q", t): {} for t in range(4)}
        # preamble: window-0 projections run inline (PE-dense, ACT idle)
        load_w(wq_s, wq_d)
        h, gs = proj_qk_ops(wq_s, xq_d, 0, "qts", holders[("q", 0)])
        for op in h + [o for g in gs for o in g]:
            op()
        load_w(wk_s, wk_d)
        h, gs = proj_qk_ops(wk_s, xk_d, 0, "kts", {})
        for op in h + [o for g in gs for o in g]:
            op()
        load_w(wv_s, wv_d)
        h, gs = proj_v_ops(0)
        for op in h + [o for g in gs for o in g]:
            op()
        for j in range(4):
            nc.sync.dma_start(wo_s[:, j, :], wo_d[j * 128:(j + 1) * 128, :])
        # q(1) head issues its DMAs during the tail of the preamble
        q1h, q1g = proj_qk_ops(wq_s, xq_d, 1, "qts", holders[("q", 1)])
        for op in q1h:
            op()

        prev_ctxn = None
        next_qg = q1g
        for t4 in range(4):
            ctxn = pctxn.tile([128, 4, 512], bf16, tag="ctxn", name="ctxn")
            fillers = deque()
            if t4 < 2:
                t = t4 + 1
                kh, kg = proj_qk_ops(wk_s, xk_d, t, "kts", {})
                vh, vg = proj_v_ops(t)
                for g in next_qg[0:3]:
                    fillers.extend(g)
                fillers.extend(kh)
                fillers.extend(next_qg[3])
                for g in kg[0:3]:
                    fillers.extend(g)
                fillers.extend(vh)
                fillers.extend(kg[3])
                for g in vg:
                    fillers.extend(g)
                if t4 == 1:
                    fillers.extend(outproj_ops(0, prev_ctxn))
                # next window's q head at the end of this window
                qh, qg = proj_qk_ops(wq_s, xq_d, t + 1, "qts",
                                     holders[("q", t + 1)])
                fillers.extend(qh)
                next_qg = qg
            elif t4 == 2:
                # q(3) and v(3) bodies must finish inside window 2
                # (qts(3) needed at w3 step 0, vext[12..15] at step 12);
                # k(3) head issues here, its body runs inside window 3.
                vh, vg = proj_v_ops(3)
                for g in next_qg[0:3]:
                    fillers.extend(g)
                fillers.extend(vh)
                fillers.extend(next_qg[3])
                for g in vg:
                    fillers.extend(g)
                fillers.extend(outproj_ops(1, prev_ctxn))
                k3h, k3g = proj_qk_ops(wk_s, xk_d, 3, "kts", {})
                fillers.extend(k3h)
            else:
                # kts[3] j-group hp is first needed at (3, hp, ki=12);
                # interleave k3 groups ahead of each deadline.
                op2 = outproj_ops(2, prev_ctxn)
                fillers.extend(k3g[0])
                fillers.extend(op2[0:5])
                fillers.extend(k3g[1])
                fillers.extend(op2[5:10])
                fillers.extend(k3g[2])
                fillers.extend(op2[10:15])
                fillers.extend(k3g[3])
                fillers.extend(op2[15:40])
            state["steps"] = 16 * (t4 + 1)
            qts_cur = holders[("q", t4)]["dst"]
            for hp in range(4):
                attention_hp(t4, qts_cur, ctxn, hp, fillers)
            drain(fillers, len(fillers))
            prev_ctxn = ctxn
        while pending:
            attnv_pop()
        outproj_tail(3, prev_ctxn)

    nc.compile()
    return nc


def _numpy_fallback(query, key, value, mask, causal_mask, Wq, Wk, Wv, Wo, bo):
    q = (query @ Wq.T).reshape(B, T, H, D).transpose(0, 2, 1, 3)
    k = (key @ Wk.T).reshape(B, T, H, D).transpose(0, 2, 1, 3)
    v = (value @ Wv.T).reshape(B, T, H, D).transpose(0, 2, 1, 3)
    out = np.zeros((B, H, T, D), np.float32)
    for b in range(B):
        for h in range(H):
            s = (q[b, h] @ k[b, h].T) / np.sqrt(np.float32(D))
            s = np.where(mask[b, 0, 0][None, :] == 0, -np.inf, s)
            if causal_mask:
                tri = np.tril(np.ones((T, T), bool))
                s = np.where(tri, s, -np.inf)
            s = s - s.max(axis=-1, keepdims=True)
            e = np.exp(s)
            a = e / e.sum(axis=-1, keepdims=True)
            out[b, h] = a @ v[b, h]
    out = out.transpose(0, 2, 1, 3).reshape(B, T, C)
    return out @ Wo.T + bo


def _in_maps(query, key, value, Wq, Wk, Wv, Wo):
    cdiag = (np.arange(128)[:, None] <= np.arange(128)[None, :]
             ).astype(ml_dtypes.bfloat16)
    cd2 = np.concatenate([cdiag, cdiag], axis=1)   # [128, 256]
    in_maps = []
    for core in range(8):
        b, g = core // 2, core % 2
        hs = g * HD
        in_maps.append({
            "xqT": np.ascontiguousarray(query[b].T).astype(ml_dtypes.bfloat16),
            "xkT": np.ascontiguousarray(key[b].T).astype(ml_dtypes.bfloat16),
            "xvT": np.ascontiguousarray(value[b].T).astype(ml_dtypes.bfloat16),
            "wq": np.ascontiguousarray(Wq[hs:hs + HD, :].T).astype(ml_dtypes.bfloat16),
            "wk": np.ascontiguousarray(Wk[hs:hs + HD, :].T).astype(ml_dtypes.bfloat16),
            "wv": np.ascontiguousarray(Wv[hs:hs + HD, :].T).astype(ml_dtypes.bfloat16),
            "wo": np.ascontiguousarray(Wo[:, hs:hs + HD].T).astype(ml_dtypes.bfloat16),
            "cdiag": cd2,
        })
    return in_maps


def kernel(**inputs):
    from concourse import bass_utils

    inp = {k: np.asarray(v) for k, v in inputs.items()}
    query, key, value = inp["query"], inp["key"], inp["value"]
    Wq, Wk, Wv, Wo, bo = inp["Wq"], inp["Wk"], inp["Wv"], inp["Wo"], inp["bo"]
    mask, causal_mask = inp["mask"], int(inp["causal_mask"])

    if (mask == 0).any() or causal_mask != 1:
        return _numpy_fallback(
            query.astype(np.float32), key.astype(np.float32),
            value.astype(np.float32), mask, causal_mask,
            Wq.astype(np.float32), Wk.astype(np.float32),
            Wv.astype(np.float32), Wo.astype(np.float32),
            bo.astype(np.float32))

    if "nc" not in _CACHE:
        _CACHE["nc"] = _build()
    nc = _CACHE["nc"]

    in_maps = _in_maps(query, key, value, Wq, Wk, Wv, Wo)
    res = bass_utils.run_bass_kernel_spmd(nc, in_maps, core_ids=list(range(8)))
    out = np.zeros((B, T, C), np.float32)
    for core in range(8):
        out[core // 2] += np.asarray(res.results[core]["y"], np.float32)
    out += bo.astype(np.float32)
    return out


def run_traced(tmpdir=None, **inputs):
    """Profiled run (test harness helper): returns BassKernelResults with
    exec_time_ns/trace populated when the axon NTFF hook is available."""
    from concourse import bass_utils

    inp = {k: np.asarray(v) for k, v in inputs.items()}
    if "nc" not in _CACHE:
        _CACHE["nc"] = _build()
    nc = _CACHE["nc"]
    in_maps = _in_maps(inp["query"], inp["key"], inp["value"],
                       inp["Wq"], inp["Wk"], inp["Wv"], inp["Wo"])
    return bass_utils.run_bass_kernel_spmd(
        nc, in_maps, core_ids=list(range(8)), trace=True, tmpdir=tmpdir)
